# revision 24
# baseline (speedup 1.0000x reference)
"""Trainium2 Bass kernel for the DiseaseGNN problem (2x SAGEConv + edge MLP).

Strategy (8 NeuronCores, SPMD):
  - Edges sorted by dst; core k owns dst range [12500k, 12500(k+1)).
  - Aggregation = one-hot matmuls accumulated in PSUM per 128-node window
    (no scatter needed). Counts ride along as a ones-column in the gathered
    message tile.
  - Per-edge gathers via the dma_gather extended instruction (int16 indices);
    tables are laid out as 8 blocks of 12544 padded rows (100352 total) and
    split into 4 chunks of 25088 rows so local indices fit in int16.
  - h = relu(W_l @ mean + W_r @ h_prev) computed in node space per window.
  - Classifier folded into node space: u = h2@wc1a.T + bc1, v = h2@wc1b.T,
    hidden = relu(u[src] + v[dst]), out = hidden@wc2.T + bc2, with u and v
    packed as one [u|v] per-node table: the AllGathered copy serves u[src],
    the local copy serves v[dst] (dst is always core-local).
  - x is shipped as one 1.6MB block per core and AllGathered + expanded to
    the 256-byte-row gather table on device.
"""
import sys
import numpy as np

for _p in ('/opt/trn_rl_repo',):
    if _p not in sys.path:
        sys.path.insert(0, _p)

import concourse.bass as bass
import concourse.bacc as bacc
import concourse.mybir as mybir
import concourse.tile as tile
from concourse.bass_utils import run_bass_kernel_spmd

f32 = mybir.dt.float32
fp16 = mybir.dt.float16
i16 = mybir.dt.int16
i8 = mybir.dt.int8

N = 100000
E = 1600000
NCORES = 8
NS = N // NCORES            # 12500 nodes per core
W = 128                      # node window
NWIN = (NS + W - 1) // W     # 98 windows per core (last partial)
BLK = NWIN * W               # 12544 padded table rows per core block
TROWS = NCORES * BLK         # 100352
NCHUNK = 4
CHUNK = TROWS // NCHUNK      # 25088 (< 32767 so int16 indices work)
GRP = 2                      # windows per gather group
NGRP = NWIN // GRP           # 49

TRACE = False
LAST_EXEC_TIME_NS = None
LAST_RUN_WALL_NS = None
PHASES = 3                   # debug: 1=layer1 only, 2=+layer2, 3=full
Q4 = True                    # debug: rotate gathers across 4 SWDGE queues

_NC_CACHE = {}

RELU = mybir.ActivationFunctionType.Relu
IDENT = mybir.ActivationFunctionType.Identity
EQ = mybir.AluOpType.is_equal
MUL = mybir.AluOpType.mult


def _build(tpc, phases=3, q4=True):
    TPW = NCHUNK * tpc                 # tiles per window
    CALL = GRP * tpc * W               # idx per (group, chunk) gather call
    VCALL = GRP * TPW * W              # idx per group v gather call
    SLOTS_COLS = NWIN * TPW * 2        # out2 cols

    nc = bacc.Bacc(num_swdge_queues=4)

    xc = nc.declare_dram_parameter("xc", [BLK, 64], fp16, isOutput=False)
    xT = nc.declare_dram_parameter("xT", [64, BLK], fp16, isOutput=False)
    src_w = nc.declare_dram_parameter("src_w", [128, NGRP * (CALL // 16)], i16, isOutput=False)
    dst_w = nc.declare_dram_parameter("dst_w", [128, NWIN * TPW], i8, isOutput=False)
    v_w = nc.declare_dram_parameter("v_w", [128, NGRP * (CALL // 16)], i16, isOutput=False)
    w1lT = nc.declare_dram_parameter("w1lT", [64, 128], fp16, isOutput=False)
    w1rT = nc.declare_dram_parameter("w1rT", [64, 128], fp16, isOutput=False)
    w2lT = nc.declare_dram_parameter("w2lT", [128, 64], fp16, isOutput=False)
    w2rT = nc.declare_dram_parameter("w2rT", [128, 64], fp16, isOutput=False)
    wc1aT = nc.declare_dram_parameter("wc1aT", [64, 64], fp16, isOutput=False)
    wc1bT = nc.declare_dram_parameter("wc1bT", [64, 64], fp16, isOutput=False)
    wc2T = nc.declare_dram_parameter("wc2T", [64, 2], fp16, isOutput=False)
    b1l_in = nc.declare_dram_parameter("b1l", [128, 1], f32, isOutput=False)
    b2l_in = nc.declare_dram_parameter("b2l", [64, 1], f32, isOutput=False)
    bc1_in = nc.declare_dram_parameter("bc1", [64, 1], f32, isOutput=False)
    bc2_in = nc.declare_dram_parameter("bc2", [128, 2], f32, isOutput=False)
    iota_in = nc.declare_dram_parameter("iota_in", [128, 128], i8, isOutput=False)
    ident_in = nc.declare_dram_parameter("ident_in", [128, 128], fp16, isOutput=False)
    out2 = nc.declare_dram_parameter("out2", [128, SLOTS_COLS], f32, isOutput=True)
    if phases < 3:
        out_h1 = nc.declare_dram_parameter("out_h1", [128, BLK], fp16, isOutput=True)
        out_u = nc.declare_dram_parameter("out_u", [BLK, 128], fp16, isOutput=True)
        out_v = nc.declare_dram_parameter("out_v", [BLK, 128], fp16, isOutput=True)

    xc_b = nc.dram_tensor("xc_bounce", [BLK, 64], fp16)
    xc_full = nc.dram_tensor("xc_full", [TROWS, 64], fp16, addr_space="Shared")
    xp = nc.dram_tensor("xp_int", [TROWS, 128], fp16)
    h1_local = nc.dram_tensor("h1_local", [BLK, 128], fp16)
    h1_full = nc.dram_tensor("h1_full", [TROWS, 128], fp16, addr_space="Shared")
    u_local = nc.dram_tensor("u_local", [BLK, 128], fp16)
    u_full = nc.dram_tensor("u_full", [TROWS, 128], fp16, addr_space="Shared")
    v_local = nc.dram_tensor("v_local", [BLK, 128], fp16)

    def load_const(pool, shape, dt, param):
        t = pool.tile(shape, dt, tag=param.name)
        nc.sync.dma_start(out=t[:], in_=param[:])
        return t

    with tile.TileContext(nc) as tc:
        with (
            tc.tile_pool(name="const", bufs=1) as const,
            tc.tile_pool(name="resident", bufs=1) as res,
        ):
            iota_sb = load_const(const, [128, 128], i8, iota_in)
            ident_sb = load_const(const, [128, 128], fp16, ident_in)
            w1lT_sb = load_const(const, [64, 128], fp16, w1lT)
            w1rT_sb = load_const(const, [64, 128], fp16, w1rT)
            w2lT_sb = load_const(const, [128, 64], fp16, w2lT)
            w2rT_sb = load_const(const, [128, 64], fp16, w2rT)
            wc1aT_sb = load_const(const, [64, 64], fp16, wc1aT)
            wc1bT_sb = load_const(const, [64, 64], fp16, wc1bT)
            wc2T_sb = load_const(const, [64, 2], fp16, wc2T)
            b1l_sb = load_const(const, [128, 1], f32, b1l_in)
            b2l_sb = load_const(const, [64, 1], f32, b2l_in)
            bc1_sb = load_const(const, [64, 1], f32, bc1_in)
            bc2_sb = load_const(const, [128, 2], f32, bc2_in)
            xT_sb = load_const(res, [64, BLK], fp16, xT)
            h1T_all = res.tile([128, BLK], fp16)
            recip_all = res.tile([128, NWIN], f32)

            HCALL = tpc * W                      # 640 idx per (window, chunk) call
            HC16 = HCALL // 16

            def load_idx_group(idxp, g):
                """One banded [128, 80] idx load per group: queue c's Q7 pair
                reads partitions 32c..32c+31, so chunk c's indices live in
                that band and one tile serves all four chunk gathers."""
                idx_t = idxp.tile([128, CALL // 16], i16, tag="idx")
                nc.sync.dma_start(
                    out=idx_t[:],
                    in_=src_w[:, g * (CALL // 16):(g + 1) * (CALL // 16)])
                return idx_t

            def gather_win(idx_t, msgp, wi, table):
                """Per-(window, chunk) gathers: 640 idx = 41 ring descs, under
                the ~64-desc SWDGE carveout ring limit."""
                msgs = []
                for c in range(NCHUNK):
                    m = msgp.tile([128, tpc, 128], fp16, tag=f"msg{c}")
                    nc.gpsimd.dma_gather(
                        out_ap=m[:], in_ap=table[c * CHUNK:(c + 1) * CHUNK, :],
                        idxs_ap=idx_t[:, wi * HC16:(wi + 1) * HC16],
                        num_idxs=HCALL, num_idxs_reg=HCALL,
                        elem_size=128, queue_num=c)
                    msgs.append(m)
                return msgs

            def load_dst(dstp, g):
                dst_t = dstp.tile([128, GRP, TPW, 1], i8)
                nc.sync.dma_start(
                    out=dst_t[:],
                    in_=dst_w[:, g * GRP * TPW:(g + 1) * GRP * TPW].rearrange(
                        "p (a b o) -> p a b o", a=GRP, o=1))
                return dst_t

            def onehot_for(ohp, dst_t, wi):
                oh = ohp.tile([128, TPW, 128], fp16)
                nc.vector.tensor_tensor(
                    out=oh[:],
                    in0=dst_t[:, wi].to_broadcast([128, TPW, 128]),
                    in1=iota_sb[:].rearrange("p (o q) -> p o q", o=1).to_broadcast([128, TPW, 128]),
                    op=EQ)
                return oh

            # AllGather each core's x block, then expand to the 256B-row
            # gather table (cols 0:64 = x, col 64 = 1.0) on device; ships
            # 1.6MB instead of 12.8MB of x per core.
            with tc.tile_pool(name="xcb", bufs=1) as xcb_p:
                xcb_t = xcb_p.tile([128, NWIN, 64], fp16)
                nc.sync.dma_start(
                    out=xcb_t[:],
                    in_=xc[:].rearrange("(a p) d -> p a d", p=128))
                nc.sync.dma_start(
                    out=xc_b[:].rearrange("(a p) d -> p a d", p=128),
                    in_=xcb_t[:])
            tc.strict_bb_all_engine_barrier()
            with tc.tile_critical():
                with nc.semaphore() as cc_sem:
                    nc.gpsimd.collective_compute(
                        "AllGather", mybir.AluOpType.bypass,
                        ins=[xc_b[:]], outs=[xc_full[:]],
                        replica_groups=[list(range(NCORES))],
                    ).then_inc(cc_sem, 1)
                    nc.gpsimd.wait_ge(cc_sem, 1)
            tc.strict_bb_all_engine_barrier()
            with tc.tile_pool(name="expand", bufs=2) as exp_p:
                ones_t = exp_p.tile([128, NWIN, 1], fp16, tag="ones")
                nc.vector.memset(ones_t[:], 1.0)
                for blk_i in range(NCORES):
                    r0 = blk_i * BLK
                    t = exp_p.tile([128, NWIN, 64], fp16, tag="xstripe")
                    nc.sync.dma_start(
                        out=t[:],
                        in_=xc_full[r0:r0 + BLK, :].rearrange("(a p) d -> p a d", p=128))
                    nc.sync.dma_start(
                        out=xp[r0:r0 + BLK, 0:64].rearrange("(a p) d -> p a d", p=128),
                        in_=t[:])
                    nc.sync.dma_start(
                        out=xp[r0:r0 + BLK, 64:65].rearrange("(a p) d -> p a d", p=128),
                        in_=ones_t[:])
            tc.strict_bb_all_engine_barrier()

            # ---------------- Phase A: layer 1 ----------------
            with (
                tc.tile_pool(name="idxA", bufs=3) as idxp,
                tc.tile_pool(name="msgA", bufs=3) as msgp,
                tc.tile_pool(name="dstA", bufs=3) as dstp,
                tc.tile_pool(name="ohA", bufs=3) as ohp,
                tc.tile_pool(name="smallA", bufs=4) as smp,
                tc.tile_pool(name="psAggA", bufs=2, space="PSUM") as psagg,
                tc.tile_pool(name="psTrA", bufs=3, space="PSUM") as pstr,
                tc.tile_pool(name="psHA", bufs=2, space="PSUM") as psh,
            ):
                for g in range(NGRP):
                    dst_t = load_dst(dstp, g)
                    gidx = load_idx_group(idxp, g)
                    for wi in range(GRP):
                        w = g * GRP + wi
                        msgs = gather_win(gidx, msgp, wi, xp)
                        oh = onehot_for(ohp, dst_t, wi)
                        agg = psagg.tile([128, 65], f32)
                        for c in range(NCHUNK):
                            for t in range(tpc):
                                nc.tensor.matmul(
                                    out=agg[:], lhsT=oh[:, c * tpc + t, :],
                                    rhs=msgs[c][:, t, :65],
                                    start=(c == 0 and t == 0),
                                    stop=(c == NCHUNK - 1 and t == tpc - 1))
                        cntm = smp.tile([128, 1], f32, tag="cnt")
                        nc.vector.tensor_scalar_max(cntm[:], agg[:, 64:65], 1.0)
                        nc.vector.reciprocal(recip_all[:, w:w + 1], cntm[:])
                        mean = smp.tile([128, 64], fp16, tag="mean")
                        nc.vector.tensor_tensor(
                            out=mean[:], in0=agg[:, :64],
                            in1=recip_all[:, w:w + 1].to_broadcast([128, 64]), op=MUL)
                        meanT_ps = pstr.tile([64, 128], fp16, tag="tr")
                        nc.tensor.transpose(meanT_ps[:], mean[:], ident_sb[:])
                        meanT = smp.tile([64, 128], fp16, tag="meanTs")
                        nc.vector.tensor_copy(meanT[:], meanT_ps[:])
                        h1ps = psh.tile([128, 128], f32, tag="h1")
                        nc.tensor.matmul(h1ps[:], lhsT=w1lT_sb[:], rhs=meanT[:], start=True, stop=False)
                        nc.tensor.matmul(h1ps[:], lhsT=w1rT_sb[:], rhs=xT_sb[:, w * 128:(w + 1) * 128],
                                         start=False, stop=True)
                        nc.scalar.activation(out=h1T_all[:, w * 128:(w + 1) * 128], in_=h1ps[:],
                                             func=RELU, bias=b1l_sb[:])
                        h1nm_ps = pstr.tile([128, 128], fp16, tag="tr")
                        nc.tensor.transpose(h1nm_ps[:], h1T_all[:, w * 128:(w + 1) * 128], ident_sb[:])
                        h1nm = smp.tile([128, 128], fp16, tag="h1nms")
                        nc.vector.tensor_copy(h1nm[:], h1nm_ps[:])
                        nc.sync.dma_start(out=h1_local[w * 128:(w + 1) * 128, :], in_=h1nm[:])

            if phases < 3:
                nc.sync.dma_start(out=out_h1[:], in_=h1T_all[:])
            if phases >= 2:
                tc.strict_bb_all_engine_barrier()
                with tc.tile_critical():
                    with nc.semaphore() as cc_sem:
                        nc.gpsimd.collective_compute(
                            "AllGather", mybir.AluOpType.bypass,
                            ins=[h1_local[:]], outs=[h1_full[:]],
                            replica_groups=[list(range(NCORES))],
                        ).then_inc(cc_sem, 1)
                        nc.gpsimd.wait_ge(cc_sem, 1)
                tc.strict_bb_all_engine_barrier()

                # ------------- Phase B: layer 2 + u/v tables -------------
                with (
                    tc.tile_pool(name="idxB", bufs=3) as idxp,
                    tc.tile_pool(name="msgB", bufs=3) as msgp,
                    tc.tile_pool(name="dstB", bufs=3) as dstp,
                    tc.tile_pool(name="ohB", bufs=3) as ohp,
                    tc.tile_pool(name="smallB", bufs=4) as smp,
                    tc.tile_pool(name="psAggB", bufs=2, space="PSUM") as psagg,
                    tc.tile_pool(name="psTrB", bufs=3, space="PSUM") as pstr,
                    tc.tile_pool(name="psHB", bufs=3, space="PSUM") as psh,
                ):
                    for g in range(NGRP):
                        dst_t = load_dst(dstp, g)
                        gidx = load_idx_group(idxp, g)
                        for wi in range(GRP):
                            w = g * GRP + wi
                            msgs = gather_win(gidx, msgp, wi, h1_full)
                            oh = onehot_for(ohp, dst_t, wi)
                            agg2 = psagg.tile([128, 128], f32)
                            for c in range(NCHUNK):
                                for t in range(tpc):
                                    nc.tensor.matmul(
                                        out=agg2[:], lhsT=oh[:, c * tpc + t, :],
                                        rhs=msgs[c][:, t, :],
                                        start=(c == 0 and t == 0),
                                        stop=(c == NCHUNK - 1 and t == tpc - 1))
                            mean2 = smp.tile([128, 128], fp16, tag="mean2")
                            nc.vector.tensor_tensor(
                                out=mean2[:], in0=agg2[:],
                                in1=recip_all[:, w:w + 1].to_broadcast([128, 128]), op=MUL)
                            mean2T_ps = pstr.tile([128, 128], fp16, tag="tr")
                            nc.tensor.transpose(mean2T_ps[:], mean2[:], ident_sb[:])
                            mean2T = smp.tile([128, 128], fp16, tag="m2Ts")
                            nc.vector.tensor_copy(mean2T[:], mean2T_ps[:])
                            h2ps = psh.tile([64, 128], f32, tag="h")
                            nc.tensor.matmul(h2ps[:], lhsT=w2lT_sb[:], rhs=mean2T[:], start=True, stop=False)
                            nc.tensor.matmul(h2ps[:], lhsT=w2rT_sb[:], rhs=h1T_all[:, w * 128:(w + 1) * 128],
                                             start=False, stop=True)
                            h2T = smp.tile([64, 128], fp16, tag="h2T")
                            nc.scalar.activation(out=h2T[:], in_=h2ps[:], func=RELU, bias=b2l_sb[:])
                            ups = psh.tile([64, 128], f32, tag="h")
                            nc.tensor.matmul(ups[:], lhsT=wc1aT_sb[:], rhs=h2T[:], start=True, stop=True)
                            uT = smp.tile([64, 128], fp16, tag="uT")
                            nc.scalar.activation(out=uT[:], in_=ups[:], func=IDENT, bias=bc1_sb[:])
                            vps = psh.tile([64, 128], f32, tag="h")
                            nc.tensor.matmul(vps[:], lhsT=wc1bT_sb[:], rhs=h2T[:], start=True, stop=True)
                            vT = smp.tile([64, 128], fp16, tag="vT")
                            nc.vector.tensor_copy(vT[:], vps[:])
                            unm_ps = pstr.tile([128, 64], fp16, tag="tr")
                            nc.tensor.transpose(unm_ps[:], uT[:], ident_sb[0:64, 0:64])
                            unm = smp.tile([128, 64], fp16, tag="unms")
                            nc.vector.tensor_copy(unm[:], unm_ps[:])
                            nc.sync.dma_start(out=u_local[w * 128:(w + 1) * 128, 0:64], in_=unm[:])
                            vnm_ps = pstr.tile([128, 64], fp16, tag="tr")
                            nc.tensor.transpose(vnm_ps[:], vT[:], ident_sb[0:64, 0:64])
                            vnm = smp.tile([128, 64], fp16, tag="vnms")
                            nc.vector.tensor_copy(vnm[:], vnm_ps[:])
                            nc.sync.dma_start(out=u_local[w * 128:(w + 1) * 128, 64:128], in_=vnm[:])

            if phases >= 3:
                tc.strict_bb_all_engine_barrier()
                with tc.tile_critical():
                    with nc.semaphore() as cc_sem:
                        nc.gpsimd.collective_compute(
                            "AllGather", mybir.AluOpType.bypass,
                            ins=[u_local[:]], outs=[u_full[:]],
                            replica_groups=[list(range(NCORES))],
                        ).then_inc(cc_sem, 1)
                        nc.gpsimd.wait_ge(cc_sem, 1)
                tc.strict_bb_all_engine_barrier()

                # ------------------ Phase C: classifier ------------------
                with (
                    tc.tile_pool(name="idxC", bufs=3) as idxp,
                    tc.tile_pool(name="gC", bufs=3) as gp,
                    tc.tile_pool(name="hC", bufs=4) as hp,
                    tc.tile_pool(name="stripC", bufs=3) as stp,
                    tc.tile_pool(name="psC", bufs=4, space="PSUM") as psc,
                    tc.tile_pool(name="psCT", bufs=3, space="PSUM") as psct,
                ):
                    HCALL = tpc * W
                    HC16 = HCALL // 16
                    for g in range(NGRP):
                        strip = stp.tile([128, GRP * TPW * 2], f32)
                        uidx_g = idxp.tile([128, CALL // 16], i16, tag="uidx")
                        nc.sync.dma_start(
                            out=uidx_g[:],
                            in_=src_w[:, g * (CALL // 16):(g + 1) * (CALL // 16)])
                        vidx_g = idxp.tile([128, CALL // 16], i16, tag="vidx")
                        nc.sync.dma_start(
                            out=vidx_g[:],
                            in_=v_w[:, g * (CALL // 16):(g + 1) * (CALL // 16)])
                        for wi in range(GRP):
                            for c in range(NCHUNK):
                                ub = gp.tile([128, tpc, 128], fp16, tag=f"ubuf{c}")
                                nc.gpsimd.dma_gather(
                                    out_ap=ub[:], in_ap=u_full[c * CHUNK:(c + 1) * CHUNK, :],
                                    idxs_ap=uidx_g[:, wi * HC16:(wi + 1) * HC16],
                                    num_idxs=HCALL, num_idxs_reg=HCALL,
                                    elem_size=128, queue_num=c)
                                vb = gp.tile([128, tpc, 128], fp16, tag=f"vbuf{c}")
                                nc.gpsimd.dma_gather(
                                    out_ap=vb[:], in_ap=u_local[:],
                                    idxs_ap=vidx_g[:, wi * HC16:(wi + 1) * HC16],
                                    num_idxs=HCALL, num_idxs_reg=HCALL,
                                    elem_size=128, queue_num=c)
                                for t in range(tpc):
                                    j = wi * TPW + c * tpc + t
                                    hsum = hp.tile([128, 64], fp16, tag="hsum")
                                    nc.vector.tensor_add(
                                        hsum[:],
                                        ub[:, t, 0:64],
                                        vb[:, t, 64:128])
                                    hid = hp.tile([128, 64], fp16, tag="hid")
                                    nc.scalar.activation(out=hid[:], in_=hsum[:], func=RELU)
                                    hT_ps = psct.tile([64, 128], fp16, tag="ct")
                                    nc.tensor.transpose(hT_ps[:], hid[:], ident_sb[:])
                                    hT = hp.tile([64, 128], fp16, tag="hT")
                                    nc.vector.tensor_copy(hT[:], hT_ps[:])
                                    ops = psc.tile([128, 2], f32)
                                    nc.tensor.matmul(ops[:], lhsT=hT[:], rhs=wc2T_sb[:],
                                                     start=True, stop=True)
                                    nc.vector.tensor_add(strip[:, j * 2:(j + 1) * 2], ops[:], bc2_sb[:])
                        nc.sync.dma_start(
                            out=out2[:, g * GRP * TPW * 2:(g + 1) * GRP * TPW * 2], in_=strip[:])

    nc.compile()
    return nc


def _get_nc(tpc):
    key = (tpc, PHASES, Q4)
    if key not in _NC_CACHE:
        _NC_CACHE[key] = _build(tpc, PHASES, Q4)
    return _NC_CACHE[key]


def _prep(x, edge_index, w1l, b1l, w1r, w2l, b2l, w2r, wc1, bc1, wc2, bc2):
    x = np.asarray(x, dtype=np.float32)
    ei = np.asarray(edge_index)
    src = ei[0].astype(np.int64)
    dst = ei[1].astype(np.int64)
    e_tot = src.shape[0]

    core_of = (dst // NS).astype(np.int64)
    win_of = ((dst % NS) // W).astype(np.int64)
    dloc = ((dst % NS) % W).astype(np.int8)
    prow_src = (src // NS) * BLK + (src % NS)
    chunk_of = prow_src // CHUNK
    gkey = (core_of * NWIN + win_of) * NCHUNK + chunk_of
    perm = np.argsort(gkey, kind='stable')
    gk_s = gkey[perm]
    counts = np.bincount(gkey, minlength=NCORES * NWIN * NCHUNK)
    tpc = max(5, int(np.ceil(counts.max() / W)))
    TPW = NCHUNK * tpc
    SLOTS = NWIN * TPW * W
    CALL = GRP * tpc * W
    VCALL = GRP * TPW * W

    starts = np.zeros(NCORES * NWIN * NCHUNK + 1, np.int64)
    np.cumsum(counts, out=starts[1:])
    pos_in_group = np.arange(e_tot) - starts[gk_s]
    k_p = gk_s // (NWIN * NCHUNK)
    wc_p = gk_s % (NWIN * NCHUNK)
    slot = wc_p * (tpc * W) + pos_in_group

    src16 = np.zeros((NCORES, SLOTS), np.int16)
    dstloc = np.full((NCORES, SLOTS), -1, np.int8)
    v16 = np.zeros((NCORES, SLOTS), np.int16)
    orig = np.full((NCORES, SLOTS), -1, np.int64)
    src16[k_p, slot] = (prow_src - chunk_of * CHUNK)[perm].astype(np.int16)
    dstloc[k_p, slot] = dloc[perm]
    v16[k_p, slot] = (dst % NS)[perm].astype(np.int16)
    orig[k_p, slot] = perm

    def wrap16(a):
        # [..., n] -> [..., 16, n//16]: idx j at (j%16, j//16)
        sh = a.shape[:-1]
        n = a.shape[-1]
        return a.reshape(sh + (n // 16, 16)).swapaxes(-1, -2)

    def band_pack(blocks):
        # blocks [NCORES, NGRP, NCHUNK, CALL]: chunk c -> partitions
        # 32c..32c+31 (wrapped idx replicated into both 16-rows)
        out = np.zeros((NCORES, 128, NGRP * (CALL // 16)), np.int16)
        wr = wrap16(blocks)                   # [k, g, c, 16, CALL//16]
        for c in range(NCHUNK):
            band = wr[:, :, c].transpose(0, 2, 1, 3).reshape(
                NCORES, 16, NGRP * (CALL // 16))
            out[:, 32 * c:32 * c + 16] = band
            out[:, 32 * c + 16:32 * c + 32] = band
        return out

    s5 = src16.reshape(NCORES, NGRP, GRP, NCHUNK, tpc * W)
    s5 = s5.transpose(0, 1, 3, 2, 4).reshape(NCORES, NGRP, NCHUNK, CALL)
    src_w = band_pack(s5)

    d5 = dstloc.reshape(NCORES, NWIN, NCHUNK, tpc, W)
    dst_w = np.ascontiguousarray(
        d5.transpose(0, 4, 1, 2, 3).reshape(NCORES, 128, NWIN * TPW))

    v5 = v16.reshape(NCORES, NGRP, GRP, NCHUNK, tpc * W)
    v5 = v5.transpose(0, 1, 3, 2, 4).reshape(NCORES, NGRP, NCHUNK, CALL)
    v_w = band_pack(v5)

    xc = np.zeros((TROWS, 64), np.float16)
    prow_all = (np.arange(N) // NS) * BLK + (np.arange(N) % NS)
    xc[prow_all] = x.astype(np.float16)

    xT_all = np.zeros((NCORES, 64, BLK), np.float16)
    xs = x.reshape(NCORES, NS, 64).astype(np.float16)
    for k in range(NCORES):
        xT_all[k, :, :NS] = xs[k].T

    w1l = np.asarray(w1l, np.float32); w1r = np.asarray(w1r, np.float32)
    w2l = np.asarray(w2l, np.float32); w2r = np.asarray(w2r, np.float32)
    wc1 = np.asarray(wc1, np.float32); wc2 = np.asarray(wc2, np.float32)
    consts = {
        "w1lT": np.ascontiguousarray(w1l.T).astype(np.float16),
        "w1rT": np.ascontiguousarray(w1r.T).astype(np.float16),
        "w2lT": np.ascontiguousarray(w2l.T).astype(np.float16),
        "w2rT": np.ascontiguousarray(w2r.T).astype(np.float16),
        "wc1aT": np.ascontiguousarray(wc1[:, :64].T).astype(np.float16),
        "wc1bT": np.ascontiguousarray(wc1[:, 64:].T).astype(np.float16),
        "wc2T": np.ascontiguousarray(wc2.T).astype(np.float16),
        "b1l": np.asarray(b1l, np.float32).reshape(128, 1),
        "b2l": np.asarray(b2l, np.float32).reshape(64, 1),
        "bc1": np.asarray(bc1, np.float32).reshape(64, 1),
        "bc2": np.broadcast_to(np.asarray(bc2, np.float32), (128, 2)).copy(),
        "iota_in": np.broadcast_to(np.arange(128, dtype=np.int8), (128, 128)).copy(),
        "ident_in": np.eye(128, dtype=np.float16),
    }

    in_maps = []
    for k in range(NCORES):
        m = {"xc": xc[k * BLK:(k + 1) * BLK], "xT": xT_all[k], "src_w": src_w[k], "dst_w": dst_w[k],
             "v_w": v_w[k]}
        m.update(consts)
        in_maps.append(m)

    meta = {"tpc": tpc, "orig": orig, "src16": src16, "dstloc": dstloc,
            "v16": v16, "e_tot": e_tot}
    return in_maps, meta


def _unscramble(results, meta):
    tpc = meta["tpc"]; orig = meta["orig"]; e_tot = meta["e_tot"]
    TPW = NCHUNK * tpc
    out = np.zeros((e_tot, 2), np.float32)
    w_arr = np.arange(NWIN)[:, None, None]
    c_arr = np.arange(NCHUNK)[None, :, None]
    t_arr = np.arange(tpc)[None, None, :]
    colbase = (w_arr // GRP) * (GRP * TPW * 2) + ((w_arr % GRP) * TPW + c_arr * tpc + t_arr) * 2
    colbase = np.repeat(colbase.reshape(NWIN * TPW), W)
    p_arr = np.tile(np.arange(W), NWIN * TPW)
    for k in range(NCORES):
        o2 = np.asarray(results[k]["out2"])
        valid = orig[k] >= 0
        out[orig[k][valid], 0] = o2[p_arr[valid], colbase[valid]]
        out[orig[k][valid], 1] = o2[p_arr[valid], colbase[valid] + 1]
    return out


def kernel(**inputs):
    global LAST_EXEC_TIME_NS, LAST_RUN_WALL_NS
    in_maps, meta = _prep(**inputs)
    nc = _get_nc(meta["tpc"])
    import time as _time
    _t0 = _time.time()
    res = run_bass_kernel_spmd(nc, in_maps, list(range(NCORES)), trace=TRACE)
    LAST_RUN_WALL_NS = int((_time.time() - _t0) * 1e9)
    LAST_EXEC_TIME_NS = res.exec_time_ns
    if PHASES < 3:
        return res.results, meta
    return _unscramble(res.results, meta)
